# revision 21
# baseline (speedup 1.0000x reference)
"""Causal self-attention (B=1, S=4096, D=768, H=12) on 8 TRN2 NeuronCores.

Sharding: 4 head-groups (3 heads each) x 2 query-parity halves; no
collectives. Core c = 2*g + p handles heads [3g, 3g+3) and query rows
{r : r % 2 == p} (strided assignment balances causal work perfectly).

V3 highlights:
  - Score matmuls (contract dim 64 = half the PE) alternate the two
    64-row PE tiles: heads 0/1 live in opposite SBUF partition halves
    and head 2's K^T/Q^T are duplicated into both halves. Alternating
    row tiles dual-streams the PE (~108 ns per 512-col matmul vs 216,
    and same-tile back-to-back runs measure 427).
  - One EXP activation covers 2048 score cols (4 PSUM banks) - the
    ACT engine has ~300ns fixed cost per call.
  - Causal band truncation: diagonal blocks only compute their valid
    query range; a single [128,64] triangle mask is applied in-place.
  - Minimal head phase; K(2..7)/V(8..31)/Q(1..3)/out-proj all run as
    fillers inside the EXP-paced attention stream so the PE never
    idles (keeps the HAM clock at 2.4 GHz).

All matmuls run in bf16 (f32 PSUM accumulation); softmax exp in f32.
"""
import os

import numpy as np
import ml_dtypes

import concourse.bass as bass
import concourse.mybir as mybir
import concourse.tile as tile
from concourse import bacc
from concourse.bass_utils import run_bass_kernel_spmd

BF16 = mybir.dt.bfloat16
F32 = mybir.dt.float32
NPBF16 = ml_dtypes.bfloat16

S = 4096          # sequence length
D = 768           # model dim
HD = 64           # head dim
HL = 3            # heads per core
DL = HL * HD      # 192 local qkv cols per core
SQ = S // 2       # 2048 local queries per core
NQT = 4           # q-tiles per core
QTW = 512         # q-tile width (local queries)
NKB = S // 128    # 32 key blocks of 128
NDC = D // 128    # 6 contraction chunks of 128 over D
VW = HD + 1       # V' column stride per head (64 V cols + ones col)
SCALE = HD ** -0.5

# band packs: diagonal blocks b paired so each pack's widths sum to <=512
BAND_PACKS = ((0,), (1, 7), (2, 6), (3, 5), (4,))


def build_nc():
    nc = bacc.Bacc(None, target_bir_lowering=False)
    xT = nc.declare_dram_parameter("xT", [D, S], BF16, isOutput=False)
    xqT = nc.declare_dram_parameter("xqT", [D, SQ], BF16, isOutput=False)
    wk = nc.declare_dram_parameter("wk", [D, DL], BF16, isOutput=False)
    wq = nc.declare_dram_parameter("wq", [D, DL], BF16, isOutput=False)
    wv = nc.declare_dram_parameter("wv", [D, DL], BF16, isOutput=False)
    bkq = nc.declare_dram_parameter("bkq", [DL, 2], F32, isOutput=False)
    bv = nc.declare_dram_parameter("bv", [DL], F32, isOutput=False)
    wout = nc.declare_dram_parameter("wout", [DL, D], BF16, isOutput=False)
    mask64 = nc.declare_dram_parameter("mask64", [128, 64], BF16, isOutput=False)
    out = nc.declare_dram_parameter("out", [SQ, D], BF16, isOutput=True)

    from contextlib import ExitStack

    with tile.TileContext(nc) as tc, ExitStack() as ctx:
        persist = ctx.enter_context(tc.tile_pool(name="persist", bufs=1))
        xtp = ctx.enter_context(tc.tile_pool(name="xtp", bufs=1))
        wp = ctx.enter_context(tc.tile_pool(name="wp", bufs=1))
        pjp = ctx.enter_context(tc.tile_pool(name="pjp", bufs=1, space="PSUM"))
        psp = ctx.enter_context(tc.tile_pool(name="psp", bufs=1, space="PSUM"))
        pop = ctx.enter_context(tc.tile_pool(name="pop", bufs=2, space="PSUM"))
        ep = ctx.enter_context(tc.tile_pool(name="ep", bufs=2))
        rp = ctx.enter_context(tc.tile_pool(name="rp", bufs=2))
        osb = ctx.enter_context(tc.tile_pool(name="osb", bufs=3))

        kT01 = persist.tile([128, S], BF16)         # K^T heads 0,1
        kT2 = persist.tile([128, S], BF16)          # K^T head 2 (both halves)
        qT01 = persist.tile([128, SQ], BF16)        # Q^T heads 0,1
        qT2 = persist.tile([128, SQ], BF16)         # Q^T head 2 (both halves)
        aT01 = persist.tile([128, SQ], BF16)        # attn^T heads 0,1
        aT2 = persist.tile([64, SQ], BF16)
        vbig = persist.tile([128, NKB * HL * VW], BF16)  # V' blocks [k,195]
        bvb = persist.tile([128, DL], F32)          # bv broadcast over rows
        msk = persist.tile([128, 64], BF16)         # causal triangle r<=2c+p
        ones1 = persist.tile([1, 64], BF16)
        bkq0 = persist.tile([128, 2], F32)
        bkq1 = persist.tile([128, 2], F32)          # [0:64] and [64:128] same
        wo0 = persist.tile([128, D], BF16)
        wo1 = persist.tile([64, D], BF16)

        nc.vector.memset(vbig, 1.0)
        nc.vector.memset(ones1, 1.0)

        # x^T / xq^T land as 512-column slices holding all 6 contraction
        # chunks: tile cols = kc*512 + j. Weights land as [128, 6*DL].
        xt = [xtp.tile([128, NDC * 512], BF16, name=f"xt{n}") for n in range(8)]
        xq = [xtp.tile([128, NDC * 512], BF16, name=f"xq{t}") for t in range(NQT)]
        wk_t = wp.tile([128, NDC * DL], BF16, name="wk")
        wq_t = wp.tile([128, NDC * DL], BF16, name="wq")
        wv_t = wp.tile([128, NDC * DL], BF16, name="wv")

        xT_r = xT.rearrange("(c p) n -> p c n", p=128)      # [128, 6, 4096]
        xq_r = xqT.rearrange("(c p) n -> p c n", p=128)     # [128, 6, 2048]

        def dma_x(dst, src_r, j0):
            nc.sync.dma_start(
                out=dst.rearrange("p (c n) -> p c n", n=512),
                in_=src_r[:, :, j0:j0 + 512])

        # need-ordered input DMAs (sync queue ~0.7us issue each):
        nc.sync.dma_start(out=wk_t.rearrange("p (c m) -> p c m", m=DL),
                          in_=wk.rearrange("(c p) m -> p c m", p=128))
        dma_x(xt[0], xT_r, 0)
        nc.sync.dma_start(out=wv_t.rearrange("p (c m) -> p c m", m=DL),
                          in_=wv.rearrange("(c p) m -> p c m", p=128))
        dma_x(xt[1], xT_r, 512)
        nc.sync.dma_start(out=wq_t.rearrange("p (c m) -> p c m", m=DL),
                          in_=wq.rearrange("(c p) m -> p c m", p=128))
        dma_x(xq[0], xq_r, 0)
        dma_x(xt[2], xT_r, 1024)
        dma_x(xt[3], xT_r, 1536)
        dma_x(xq[1], xq_r, 512)
        dma_x(xt[4], xT_r, 2048)
        dma_x(xt[5], xT_r, 2560)
        dma_x(xq[2], xq_r, 1024)
        dma_x(xt[6], xT_r, 3072)
        dma_x(xt[7], xT_r, 3584)
        dma_x(xq[3], xq_r, 1536)
        # small tensors on the gpsimd queue (parallel issue path)
        nc.gpsimd.dma_start(out=bkq0, in_=bkq[0:128, :])
        nc.gpsimd.dma_start(out=bkq1[0:64, :], in_=bkq[128:DL, :])
        nc.gpsimd.dma_start(out=bkq1[64:128, :], in_=bkq[128:DL, :])
        nc.gpsimd.dma_start(out=bvb, in_=bv[:].partition_broadcast(128))
        nc.gpsimd.dma_start(out=msk, in_=mask64[:, :])
        nc.gpsimd.dma_start(out=wo0, in_=wout[0:128, :])
        nc.gpsimd.dma_start(out=wo1, in_=wout[128:DL, :])

        def kq_proj(dst01, dst2, w_t, rhs, bc, n, m):
            # dst[m-rows, cols n*512..] = W^T x^T + b  for one m-pass
            nsl = slice(n * 512, (n + 1) * 512)
            mw = 128 if m == 0 else 64
            msl = slice(0, 128) if m == 0 else slice(128, DL)
            ps = pjp.tile([128, 512], F32, name="pj", tag="pj")
            for kc in range(NDC):
                nc.tensor.matmul(
                    ps[:mw, :],
                    lhsT=w_t[:, kc * DL:(kc + 1) * DL][:, msl],
                    rhs=rhs[:, kc * 512:(kc + 1) * 512],
                    start=(kc == 0), stop=(kc == NDC - 1),
                )
            if m == 0:
                nc.vector.tensor_scalar_add(
                    out=dst01[:, nsl], in0=ps, scalar1=bkq0[:, bc:bc + 1])
            else:  # head 2: write both partition halves (dual-tile scores)
                nc.vector.tensor_scalar_add(
                    out=dst2[0:64, nsl], in0=ps[:64, :],
                    scalar1=bkq1[0:64, bc:bc + 1])
                nc.vector.tensor_scalar_add(
                    out=dst2[64:128, nsl], in0=ps[:64, :],
                    scalar1=bkq1[64:128, bc:bc + 1])

        def v_proj(kb):
            pv = pjp.tile([128, 512], F32, name="pj", tag="pj")
            n, j = kb // 4, (kb % 4) * 128
            for kc in range(NDC):
                nc.tensor.matmul(
                    pv[:, :DL], lhsT=xt[n][:, kc * 512 + j:kc * 512 + j + 128],
                    rhs=wv_t[:, kc * DL:(kc + 1) * DL],
                    start=(kc == 0), stop=(kc == NDC - 1),
                )
            # one strided add writes all 3 heads' V cols (ones col skipped)
            voff = kb * HL * VW
            dstv = vbig[:, voff:voff + HL * VW]
            dstv = dstv.rearrange("p (h vw) -> p h vw", vw=VW)[:, :, 0:HD]
            nc.vector.tensor_add(
                out=dstv,
                in0=pv[:, :DL].rearrange("p (h d) -> p h d", d=HD),
                in1=bvb.rearrange("p (h d) -> p h d", d=HD),
            )

        # per-head score/attn tiles: (lhsT source, rhs source, aT dest)
        kq_src = (
            (kT01, qT01, (0, 64)),     # head 0: always low half
            (kT01, qT01, (64, 128)),   # head 1: always high half
            (kT2, qT2, None),          # head 2: half chosen per matmul
        )
        aT_of = (aT01[0:64], aT01[64:128], aT2)

        ot_tiles = {}

        def out_proj_half(qt, ncol, pot=None, reg=0):
            # split at the PSUM-slot reuse boundary so the WAR wait on the
            # previous half's copy never stalls the PE mid-filler
            osl = slice(qt * 128, (qt + 1) * 128)
            if ncol == 0:
                ot_tiles[qt] = osb.tile([128, D], BF16, name="ot", tag="ot")
            ot = ot_tiles[qt]
            cw = 512 if ncol == 0 else 256
            csl = slice(ncol * 512, ncol * 512 + cw)
            final = pot is not None
            if not final:
                pot = pjp.tile([128, 512], F32, name="pj", tag="pj")
            psl = slice(reg * 512, reg * 512 + cw)
            nc.tensor.matmul(
                pot[:, psl], lhsT=aT01[:, osl], rhs=wo0[:, csl],
                start=True, stop=False, skip_group_check=True)
            nc.tensor.matmul(
                pot[:, psl], lhsT=aT2[:, osl], rhs=wo1[:, csl],
                start=False, stop=True, skip_group_check=True)
            nc.vector.tensor_copy(out=ot[:, csl], in_=pot[:, psl])
            if final:
                nc.gpsimd.dma_start(out=out[osl, csl], in_=ot[:, csl])
            elif ncol == 1:
                nc.gpsimd.dma_start(out=out[osl, :], in_=ot)

        def attention(t, fillers):
            def pump(k=1):
                for _ in range(k):
                    if fillers:
                        fillers.pop(0)()

            qoff = t * QTW
            last_kb = 8 * t + BAND_PACKS[-1][-1]

            # entry = (half, head, kb, psum_off, width, q_start, band)
            # pack = (entries, exp_ranges, heads_finishing)
            packs = []
            # phase A: heads 0+1 paired on alternating PE row tiles
            for kb in range(0, 8 * t, 2):
                ent = []
                for i, kbx in enumerate((kb, kb + 1)):
                    ent.append((0, 0, kbx, 1024 * i, 512, 0, False))
                    ent.append((64, 1, kbx, 1024 * i + 512, 512, 0, False))
                packs.append((ent, ((0, 2048),), ()))
            for pr_pair in (((0,), (1, 7)), ((2, 6), (3, 5))):
                ent = []
                for i, pr in enumerate(pr_pair):
                    offs = [1024 * i, 1024 * i + 512]
                    for b in pr:
                        w = 512 - 64 * b
                        for h in (0, 1):
                            ent.append((64 * h, h, 8 * t + b, offs[h], w,
                                        64 * b, True))
                            offs[h] += w
                packs.append((ent, ((0, 2048),), ()))
            packs.append((
                [(0, 0, 8 * t + 4, 0, 256, 256, True),
                 (64, 1, 8 * t + 4, 512, 256, 256, True)],
                ((0, 256), (512, 768)), (0, 1)))
            # phase B: head 2 alternating its two duplicated halves
            for kb in range(0, 8 * t, 4):
                ent = [(64 * (i % 2), 2, kb + i, 512 * i, 512, 0, False)
                       for i in range(4)]
                packs.append((ent, ((0, 2048),), ()))
            for half, prs in ((0, ((0,), (1, 7))), (64, ((2, 6), (3, 5)))):
                ent = []
                off = 0
                for pr in prs:
                    for b in pr:
                        w = 512 - 64 * b
                        ent.append((half, 2, 8 * t + b, off, w, 64 * b, True))
                        off += w
                    off = 512
                packs.append((ent, ((0, 1024),), ()))
            packs.append((
                [(0, 2, 8 * t + 4, 0, 256, 256, True)],
                ((0, 256),), (2,)))

            po_of = {}

            def emit_pv(pack, eT):
                entries, _, fin = pack
                for (_half, h, kb, off, w, qs, _band) in entries:
                    if h not in po_of:
                        # lazy: the slot's previous reader (divide of the
                        # evicted head) must already be emitted for the WAR
                        po_of[h] = pop.tile([VW, 512], F32, name="po",
                                            tag="po")
                    voff = kb * HL * VW + h * VW
                    nc.tensor.matmul(
                        po_of[h][0:VW, qs:qs + w],
                        lhsT=vbig[:, voff:voff + VW],
                        rhs=eT[:, off:off + w],
                        start=(kb == 0), stop=(kb == last_kb),
                        skip_group_check=True,
                    )
                for h in fin:
                    divide(h)

            def divide(h):
                # divide by the softmax sum (row HD of po)
                po = po_of[h]
                sums = rp.tile([1, 512], BF16, name="sums", tag="sums")
                nc.vector.tensor_copy(out=sums, in_=po[HD:VW, :])
                pb = pjp.tile([128, 512], F32, name="pj", tag="pj")
                nc.tensor.matmul(pb[0:64, :], lhsT=ones1, rhs=sums,
                                 start=True, stop=True)
                recb = rp.tile([64, 512], F32, name="recb", tag="recb")
                nc.vector.reciprocal_approx_fast(out=recb, in_=pb[0:64, :])
                nc.vector.tensor_mul(
                    out=aT_of[h][:, qoff:qoff + QTW], in0=po[0:HD, :],
                    in1=recb)

            pend = None  # (pack, eT) whose PV is not yet emitted
            for pack in packs:
                entries, exp_ranges, _fin = pack
                ps = psp.tile([128, 2048], F32, name="ps", tag="ps")
                for (half, h, kb, off, w, qs, _band) in entries:
                    kT_h, qT_h, fixed = kq_src[h]
                    if fixed is not None:
                        hsl = slice(fixed[0], fixed[1])
                    else:
                        hsl = slice(half, half + 64)
                    nc.tensor.matmul(
                        ps[:, off:off + w],
                        lhsT=kT_h[hsl, kb * 128:(kb + 1) * 128],
                        rhs=qT_h[hsl, qoff + qs:qoff + QTW],
                        start=True, stop=True,
                    )
                eT = ep.tile([128, 2048], BF16, name="eT", tag="eT")
                for (r0, r1) in exp_ranges:
                    while r0 < r1:  # ACT PSUM reads are capped at 1024 f32
                        rm = min(r1, r0 + 1024)
                        nc.scalar.activation(
                            out=eT[:, r0:rm], in_=ps[:, r0:rm],
                            func=mybir.ActivationFunctionType.Exp, scale=SCALE)
                        r0 = rm
                for (_half, _h, kb, off, w, qs, band) in entries:
                    if band:  # zero the 64 partial cols of the triangle
                        nc.vector.tensor_mul(
                            out=eT[:, off:off + 64],
                            in0=eT[:, off:off + 64], in1=msk)
                pump(1)
                if pend is not None:
                    emit_pv(*pend)
                pend = (pack, eT)
            emit_pv(*pend)
            pump(len(fillers))

        # ---- schedule: minimal head, then q-tiles t=0..3 with fillers ----
        def K(n, m):
            return lambda: kq_proj(kT01, kT2, wk_t, xt[n], 0, n, m)

        def Q(t, m):
            return lambda: kq_proj(qT01, qT2, wq_t, xq[t], 1, t, m)

        def V(kb):
            return lambda: v_proj(kb)

        def O(qt, ncol):
            return lambda: out_proj_half(qt, ncol)

        for f in [K(0, 0), K(0, 1), K(1, 0), K(1, 1)]:
            f()
        for kb in range(8):
            v_proj(kb)
        Q(0, 0)()
        Q(0, 1)()

        FILL = {
            0: [K(2, 0), K(2, 1), V(8), V(9), K(3, 0), K(3, 1),
                V(10), V(11), V(12), V(13), V(14), V(15), Q(1, 0), Q(1, 1)],
            1: [K(4, 0), K(4, 1), V(16), V(17), K(5, 0), K(5, 1),
                V(18), V(19), Q(2, 0), Q(2, 1)],
            2: [K(6, 0), K(6, 1), V(20), V(21), K(7, 0), K(7, 1),
                V(22), V(23), Q(3, 0), Q(3, 1), O(0, 0), O(0, 1),
                O(1, 0), O(1, 1), O(2, 0), O(2, 1), O(3, 0), O(3, 1)],
            3: [V(24), V(25), V(26), V(27), V(28), V(29), V(30), V(31),
                O(4, 0), O(4, 1), O(5, 0), O(5, 1), O(6, 0), O(6, 1),
                O(7, 0), O(7, 1), O(8, 0), O(8, 1), O(9, 0), O(9, 1),
                O(10, 0), O(10, 1), O(11, 0), O(11, 1)],
        }
        for t in range(NQT):
            attention(t, FILL[t])
        # last tile's out-projection: one 4-bank PSUM tile, halves rotate
        # through its bank regions so nothing serializes on the copies
        potf = psp.tile([128, 2048], F32, name="ps", tag="ps")
        for i, (qt, ncol) in enumerate(((12, 0), (13, 0), (12, 1), (14, 0),
                                        (13, 1), (15, 0), (14, 1), (15, 1))):
            out_proj_half(qt, ncol, pot=potf, reg=i % 4)

    nc.finalize()
    return nc


_NC_CACHE = {}


def _get_nc():
    if "nc" not in _NC_CACHE:
        _NC_CACHE["nc"] = build_nc()
    return _NC_CACHE["nc"]


def kernel(x, Wqkv, bqkv, Wout, bout):
    x = np.asarray(x, dtype=np.float32)
    Wqkv = np.asarray(Wqkv, dtype=np.float32)
    bqkv = np.asarray(bqkv, dtype=np.float32)
    Wout = np.asarray(Wout, dtype=np.float32)
    bout = np.asarray(bout, dtype=np.float32)
    B, S_, D_ = x.shape
    assert (B, S_, D_) == (1, S, D)
    nc = _get_nc()

    xT_np = np.ascontiguousarray(x[0].T).astype(NPBF16)          # [768, 4096]
    in_maps = []
    for c in range(8):
        g, p = c // 2, c % 2
        csl = slice(DL * g, DL * (g + 1))
        rr = np.arange(128, dtype=np.int64)[:, None]
        cc = np.arange(64, dtype=np.int64)[None, :]
        mask = (rr <= 2 * cc + p).astype(NPBF16)
        bk_h = bqkv[D + DL * g:D + DL * (g + 1)].astype(np.float32)
        bq_h = bqkv[csl].astype(np.float32)
        in_maps.append({
            "xT": xT_np,
            "xqT": np.ascontiguousarray(xT_np[:, p::2]),
            "wk": np.ascontiguousarray(Wqkv[:, D + DL * g:D + DL * (g + 1)]).astype(NPBF16),
            "wq": np.ascontiguousarray(Wqkv[:, csl]).astype(NPBF16),
            "wv": np.ascontiguousarray(Wqkv[:, 2 * D + DL * g:2 * D + DL * (g + 1)]).astype(NPBF16),
            "bkq": np.ascontiguousarray(np.stack([bk_h, bq_h], axis=1)),
            "bv": np.ascontiguousarray(bqkv[2 * D + DL * g:2 * D + DL * (g + 1)]).astype(np.float32),
            "wout": np.ascontiguousarray(Wout[csl, :]).astype(NPBF16),
            "mask64": mask,
        })

    trace = bool(int(os.environ.get("ATTN_TRACE", "0")))
    tmpdir = os.environ.get("ATTN_TMPDIR") or None
    res = run_bass_kernel_spmd(nc, in_maps, core_ids=list(range(8)), trace=trace,
                               tmpdir=tmpdir)
    if trace:
        _NC_CACHE["last_result"] = res

    out_full = np.zeros((S, D), np.float32)
    for p in range(2):
        acc = np.zeros((SQ, D), np.float32)
        for g in range(4):
            acc += res.results[2 * g + p]["out"].astype(np.float32)
        out_full[p::2] = acc
    out_full += bout.astype(np.float32)[None, :]
    return out_full[None].astype(np.float32)


# revision 27
# speedup vs baseline: 1.1489x; 1.1489x over previous
"""Causal self-attention (B=1, S=4096, D=768, H=12) on 8 TRN2 NeuronCores.

Sharding: 4 head-groups (3 heads each) x 2 query-parity halves; no
collectives. Core c = 2*g + p handles heads [3g, 3g+3) and query rows
{r : r % 2 == p} (strided assignment balances causal work perfectly).

V3 highlights:
  - Score matmuls (contract dim 64 = half the PE) alternate the two
    64-row PE tiles: heads 0/1 live in opposite SBUF partition halves
    and head 2's K^T/Q^T are duplicated into both halves. Alternating
    row tiles dual-streams the PE (~108 ns per 512-col matmul vs 216,
    and same-tile back-to-back runs measure 427).
  - One EXP activation covers 2048 score cols (4 PSUM banks) - the
    ACT engine has ~300ns fixed cost per call.
  - Causal band truncation: diagonal blocks only compute their valid
    query range; a single [128,64] triangle mask is applied in-place.
  - Minimal head phase; K(2..7)/V(8..31)/Q(1..3)/out-proj all run as
    fillers inside the EXP-paced attention stream so the PE never
    idles (keeps the HAM clock at 2.4 GHz).

All matmuls run in bf16 (f32 PSUM accumulation); softmax exp in f32.
"""
import os

import numpy as np
import ml_dtypes

import concourse.bass as bass
import concourse.mybir as mybir
import concourse.tile as tile
from concourse import bacc
from concourse.bass_utils import run_bass_kernel_spmd

BF16 = mybir.dt.bfloat16
F32 = mybir.dt.float32
NPBF16 = ml_dtypes.bfloat16

S = 4096          # sequence length
D = 768           # model dim
HD = 64           # head dim
HL = 3            # heads per core
DL = HL * HD      # 192 local qkv cols per core
SQ = S // 2       # 2048 local queries per core
NQT = 4           # q-tiles per core
QTW = 512         # q-tile width (local queries)
NKB = S // 128    # 32 key blocks of 128
NDC = D // 128    # 6 contraction chunks of 128 over D
VW = HD + 1       # V' column stride per head (64 V cols + ones col)
SCALE = HD ** -0.5

# band packs: diagonal blocks b paired so each pack's widths sum to <=512
BAND_PACKS = ((0,), (1, 7), (2, 6), (3, 5), (4,))


def build_nc():
    nc = bacc.Bacc(None, target_bir_lowering=False)
    xT = nc.declare_dram_parameter("xT", [D, S], BF16, isOutput=False)
    xqT = nc.declare_dram_parameter("xqT", [D, SQ], BF16, isOutput=False)
    wk = nc.declare_dram_parameter("wk", [D, DL], BF16, isOutput=False)
    wq = nc.declare_dram_parameter("wq", [D, DL], BF16, isOutput=False)
    wv = nc.declare_dram_parameter("wv", [D, DL], BF16, isOutput=False)
    bkq = nc.declare_dram_parameter("bkq", [DL, 2], F32, isOutput=False)
    bv = nc.declare_dram_parameter("bv", [DL], F32, isOutput=False)
    wout = nc.declare_dram_parameter("wout", [DL, D], BF16, isOutput=False)
    mask64 = nc.declare_dram_parameter("mask64", [128, 64], BF16, isOutput=False)
    out = nc.declare_dram_parameter("out", [SQ, D], BF16, isOutput=True)

    from contextlib import ExitStack

    with tile.TileContext(nc) as tc, ExitStack() as ctx:
        persist = ctx.enter_context(tc.tile_pool(name="persist", bufs=1))
        xtp = ctx.enter_context(tc.tile_pool(name="xtp", bufs=1))
        wp = ctx.enter_context(tc.tile_pool(name="wp", bufs=1))
        pjp = ctx.enter_context(tc.tile_pool(name="pjp", bufs=1, space="PSUM"))
        psp = ctx.enter_context(tc.tile_pool(name="psp", bufs=1, space="PSUM"))
        pop = ctx.enter_context(tc.tile_pool(name="pop", bufs=2, space="PSUM"))
        ep = ctx.enter_context(tc.tile_pool(name="ep", bufs=2))
        rp = ctx.enter_context(tc.tile_pool(name="rp", bufs=2))
        osb = ctx.enter_context(tc.tile_pool(name="osb", bufs=3))

        kT01 = persist.tile([128, S], BF16)         # K^T heads 0,1
        kT2 = persist.tile([128, S], BF16)          # K^T head 2 (both halves)
        qT01 = persist.tile([128, SQ], BF16)        # Q^T heads 0,1
        qT2 = persist.tile([128, SQ], BF16)         # Q^T head 2 (both halves)
        aT01 = persist.tile([128, SQ], BF16)        # attn^T heads 0,1
        aT2 = persist.tile([64, SQ], BF16)
        vbig = persist.tile([128, NKB * HL * VW], BF16)  # V' blocks [k,195]
        bvb = persist.tile([128, DL], F32)          # bv broadcast over rows
        msk = persist.tile([128, 64], BF16)         # causal triangle r<=2c+p
        ones1 = persist.tile([1, 64], BF16)
        bkq0 = persist.tile([128, 2], F32)
        bkq1 = persist.tile([128, 2], F32)          # [0:64] and [64:128] same
        wo0 = persist.tile([128, D], BF16)
        wo1 = persist.tile([64, D], BF16)

        nc.vector.memset(vbig, 1.0)
        nc.vector.memset(ones1, 1.0)

        # x^T / xq^T land as 512-column slices holding all 6 contraction
        # chunks: tile cols = kc*512 + j. Weights land as [128, 6*DL].
        xt = [xtp.tile([128, NDC * 512], BF16, name=f"xt{n}") for n in range(8)]
        xq = [xtp.tile([128, NDC * 512], BF16, name=f"xq{t}") for t in range(NQT)]
        wk_t = wp.tile([128, NDC * DL], BF16, name="wk")
        wq_t = wp.tile([128, NDC * DL], BF16, name="wq")
        wv_t = wp.tile([128, NDC * DL], BF16, name="wv")

        xT_r = xT.rearrange("(c p) n -> p c n", p=128)      # [128, 6, 4096]
        xq_r = xqT.rearrange("(c p) n -> p c n", p=128)     # [128, 6, 2048]

        def dma_x(dst, src_r, j0):
            nc.sync.dma_start(
                out=dst.rearrange("p (c n) -> p c n", n=512),
                in_=src_r[:, :, j0:j0 + 512])

        # need-ordered input DMAs (sync queue ~0.7us issue each):
        nc.sync.dma_start(out=wk_t.rearrange("p (c m) -> p c m", m=DL),
                          in_=wk.rearrange("(c p) m -> p c m", p=128))
        dma_x(xt[0], xT_r, 0)
        dma_x(xt[1], xT_r, 512)
        nc.sync.dma_start(out=wv_t.rearrange("p (c m) -> p c m", m=DL),
                          in_=wv.rearrange("(c p) m -> p c m", p=128))
        nc.sync.dma_start(out=wq_t.rearrange("p (c m) -> p c m", m=DL),
                          in_=wq.rearrange("(c p) m -> p c m", p=128))
        dma_x(xq[0], xq_r, 0)
        dma_x(xt[2], xT_r, 1024)
        dma_x(xt[3], xT_r, 1536)
        dma_x(xq[1], xq_r, 512)
        dma_x(xt[4], xT_r, 2048)
        dma_x(xt[5], xT_r, 2560)
        dma_x(xq[2], xq_r, 1024)
        dma_x(xt[6], xT_r, 3072)
        dma_x(xt[7], xT_r, 3584)
        dma_x(xq[3], xq_r, 1536)
        # small tensors on the gpsimd queue (parallel issue path)
        nc.gpsimd.dma_start(out=bkq0, in_=bkq[0:128, :])
        nc.gpsimd.dma_start(out=bkq1[0:64, :], in_=bkq[128:DL, :])
        nc.gpsimd.dma_start(out=bkq1[64:128, :], in_=bkq[128:DL, :])
        nc.gpsimd.dma_start(out=bvb, in_=bv[:].partition_broadcast(128))
        nc.gpsimd.dma_start(out=msk, in_=mask64[:, :])
        nc.gpsimd.dma_start(out=wo0, in_=wout[0:128, :])
        nc.gpsimd.dma_start(out=wo1, in_=wout[128:DL, :])

        def kq_proj(dst01, dst2, w_t, rhs, bc, n, m):
            # dst[m-rows, cols n*512..] = W^T x^T + b  for one m-pass
            nsl = slice(n * 512, (n + 1) * 512)
            mw = 128 if m == 0 else 64
            msl = slice(0, 128) if m == 0 else slice(128, DL)
            ps = pjp.tile([128, 512], F32, name="pj", tag="pj")
            for kc in range(NDC):
                nc.tensor.matmul(
                    ps[:mw, :],
                    lhsT=w_t[:, kc * DL:(kc + 1) * DL][:, msl],
                    rhs=rhs[:, kc * 512:(kc + 1) * 512],
                    start=(kc == 0), stop=(kc == NDC - 1),
                )
            if m == 0:
                nc.vector.tensor_scalar_add(
                    out=dst01[:, nsl], in0=ps, scalar1=bkq0[:, bc:bc + 1])
            else:  # head 2: write both partition halves (dual-tile scores)
                nc.vector.tensor_scalar_add(
                    out=dst2[0:64, nsl], in0=ps[:64, :],
                    scalar1=bkq1[0:64, bc:bc + 1])
                nc.vector.tensor_scalar_add(
                    out=dst2[64:128, nsl], in0=ps[:64, :],
                    scalar1=bkq1[64:128, bc:bc + 1])

        def v_proj(kb):
            pv = pjp.tile([128, 512], F32, name="pj", tag="pj")
            n, j = kb // 4, (kb % 4) * 128
            for kc in range(NDC):
                nc.tensor.matmul(
                    pv[:, :DL], lhsT=xt[n][:, kc * 512 + j:kc * 512 + j + 128],
                    rhs=wv_t[:, kc * DL:(kc + 1) * DL],
                    start=(kc == 0), stop=(kc == NDC - 1),
                )
            # one strided add writes all 3 heads' V cols (ones col skipped)
            voff = kb * HL * VW
            dstv = vbig[:, voff:voff + HL * VW]
            dstv = dstv.rearrange("p (h vw) -> p h vw", vw=VW)[:, :, 0:HD]
            nc.vector.tensor_add(
                out=dstv,
                in0=pv[:, :DL].rearrange("p (h d) -> p h d", d=HD),
                in1=bvb.rearrange("p (h d) -> p h d", d=HD),
            )

        # per-head score/attn tiles: (lhsT source, rhs source, aT dest)
        kq_src = (
            (kT01, qT01, (0, 64)),     # head 0: always low half
            (kT01, qT01, (64, 128)),   # head 1: always high half
            (kT2, qT2, None),          # head 2: half chosen per matmul
        )
        aT_of = (aT01[0:64], aT01[64:128], aT2)

        ot_tiles = {}

        def out_proj_half(qt, ncol, pot=None, reg=0):
            # split at the PSUM-slot reuse boundary so the WAR wait on the
            # previous half's copy never stalls the PE mid-filler
            osl = slice(qt * 128, (qt + 1) * 128)
            if ncol == 0:
                ot_tiles[qt] = osb.tile([128, D], BF16, name="ot", tag="ot")
            ot = ot_tiles[qt]
            cw = 512 if ncol == 0 else 256
            csl = slice(ncol * 512, ncol * 512 + cw)
            final = pot is not None
            if not final:
                pot = pjp.tile([128, 512], F32, name="pj", tag="pj")
            psl = slice(reg * 512, reg * 512 + cw)
            nc.tensor.matmul(
                pot[:, psl], lhsT=aT01[:, osl], rhs=wo0[:, csl],
                start=True, stop=False, skip_group_check=True)
            nc.tensor.matmul(
                pot[:, psl], lhsT=aT2[:, osl], rhs=wo1[:, csl],
                start=False, stop=True, skip_group_check=True)
            nc.vector.tensor_copy(out=ot[:, csl], in_=pot[:, psl])
            if final:
                nc.gpsimd.dma_start(out=out[osl, csl], in_=ot[:, csl])
            elif ncol == 1:
                nc.gpsimd.dma_start(out=out[osl, :], in_=ot)

        def attention(t, fillers):
            def pump(k=1):
                for _ in range(k):
                    if fillers:
                        fillers.pop(0)()

            qoff = t * QTW
            last_kb = 8 * t + BAND_PACKS[-1][-1]

            # entry = (half, head, kb, psum_off, width, q_start, band)
            # pack = (entries, exp_ranges, heads_finishing)
            packs = []
            # phase A: heads 0+1 paired on alternating PE row tiles
            for kb in range(0, 8 * t):
                packs.append((
                    [(0, 0, kb, 0, 512, 0, False),
                     (64, 1, kb, 512, 512, 0, False)],
                    ((0, 1024),), ()))
            for pr in BAND_PACKS[:-1]:
                ent = []
                offs = [0, 512]
                for b in pr:
                    w = 512 - 64 * b
                    for h in (0, 1):
                        ent.append((64 * h, h, 8 * t + b, offs[h], w,
                                    64 * b, True))
                        offs[h] += w
                packs.append((ent, ((0, 1024),), ()))
            packs.append((
                [(0, 0, 8 * t + 4, 0, 256, 256, True),
                 (64, 1, 8 * t + 4, 512, 256, 256, True)],
                ((0, 256), (512, 768)), (0, 1)))
            # phase B: head 2 alternating its two duplicated halves
            for kb in range(0, 8 * t, 2):
                packs.append((
                    [(0, 2, kb, 0, 512, 0, False),
                     (64, 2, kb + 1, 512, 512, 0, False)],
                    ((0, 1024),), ()))
            for i, prs in enumerate((((0,), (1, 7)), ((2, 6), (3, 5)))):
                ent = []
                for j, pr in enumerate(prs):
                    off = 512 * j
                    for b in pr:
                        w = 512 - 64 * b
                        ent.append((64 * ((2 * i + j) % 2), 2, 8 * t + b,
                                    off, w, 64 * b, True))
                        off += w
                packs.append((ent, ((0, 1024),), ()))
            packs.append((
                [(0, 2, 8 * t + 4, 0, 256, 256, True)],
                ((0, 256),), (2,)))

            po_of = {}

            def emit_pv(pack, eT):
                entries, _, fin = pack
                for (_half, h, kb, off, w, qs, _band) in entries:
                    if h not in po_of:
                        # lazy: the slot's previous reader (divide of the
                        # evicted head) must already be emitted for the WAR
                        po_of[h] = pop.tile([VW, 512], F32, name="po",
                                            tag="po")
                    voff = kb * HL * VW + h * VW
                    nc.tensor.matmul(
                        po_of[h][0:VW, qs:qs + w],
                        lhsT=vbig[:, voff:voff + VW],
                        rhs=eT[:, off:off + w],
                        start=(kb == 0), stop=(kb == last_kb),
                        skip_group_check=True,
                    )
                for h in fin:
                    divide(h)

            def divide(h):
                # divide by the softmax sum (row HD of po)
                po = po_of[h]
                sums = rp.tile([1, 512], BF16, name="sums", tag="sums")
                nc.vector.tensor_copy(out=sums, in_=po[HD:VW, :])
                pb = pjp.tile([128, 512], F32, name="pj", tag="pj")
                nc.tensor.matmul(pb[0:64, :], lhsT=ones1, rhs=sums,
                                 start=True, stop=True)
                recb = rp.tile([64, 512], F32, name="recb", tag="recb")
                nc.vector.reciprocal_approx_fast(out=recb, in_=pb[0:64, :])
                nc.vector.tensor_mul(
                    out=aT_of[h][:, qoff:qoff + QTW], in0=po[0:HD, :],
                    in1=recb)

            pend = None  # (pack, eT) whose PV is not yet emitted
            for pack in packs:
                entries, exp_ranges, _fin = pack
                ps = psp.tile([128, 1024], F32, name="ps", tag="ps", bufs=2)
                for (half, h, kb, off, w, qs, _band) in entries:
                    kT_h, qT_h, fixed = kq_src[h]
                    if fixed is not None:
                        hsl = slice(fixed[0], fixed[1])
                    else:
                        hsl = slice(half, half + 64)
                    nc.tensor.matmul(
                        ps[:, off:off + w],
                        lhsT=kT_h[hsl, kb * 128:(kb + 1) * 128],
                        rhs=qT_h[hsl, qoff + qs:qoff + QTW],
                        start=True, stop=True,
                    )
                eT = ep.tile([128, 1024], BF16, name="eT", tag="eT", bufs=3)
                for (r0, r1) in exp_ranges:
                    nc.scalar.activation(
                        out=eT[:, r0:r1], in_=ps[:, r0:r1],
                        func=mybir.ActivationFunctionType.Exp, scale=SCALE)
                for (_half, _h, kb, off, w, qs, band) in entries:
                    if band:  # zero the 64 partial cols of the triangle
                        nc.vector.tensor_mul(
                            out=eT[:, off:off + 64],
                            in0=eT[:, off:off + 64], in1=msk)
                pump(1)
                if pend is not None:
                    emit_pv(*pend)
                pend = (pack, eT)
            emit_pv(*pend)
            pump(len(fillers))

        # ---- schedule: minimal head, then q-tiles t=0..3 with fillers ----
        def K(n, m):
            return lambda: kq_proj(kT01, kT2, wk_t, xt[n], 0, n, m)

        def Q(t, m):
            return lambda: kq_proj(qT01, qT2, wq_t, xq[t], 1, t, m)

        def V(kb):
            return lambda: v_proj(kb)

        def O(qt, ncol):
            return lambda: out_proj_half(qt, ncol)

        for f in [K(0, 0), K(0, 1), K(1, 0), K(1, 1)]:
            f()
        for kb in range(8):
            v_proj(kb)
        Q(0, 0)()
        Q(0, 1)()

        # V fillers ordered by the band-block usage order of the next tile
        def Vband(t):
            return [V(8 * t + b) for b in (0, 1, 7, 2, 6, 3, 5, 4)]

        FILL = {
            0: [K(2, 0), K(2, 1), K(3, 0), K(3, 1), Q(1, 0), Q(1, 1)],
            1: Vband(1) + [K(4, 0), K(4, 1), K(5, 0), K(5, 1),
                           Q(2, 0), Q(2, 1)],
            2: Vband(2) + [K(6, 0), K(6, 1), K(7, 0), K(7, 1),
                           Q(3, 0), Q(3, 1), O(0, 0), O(0, 1)],
            3: Vband(3) + [O(1, 0), O(1, 1), O(2, 0), O(2, 1),
                           O(3, 0), O(3, 1), O(4, 0), O(4, 1),
                           O(5, 0), O(5, 1), O(6, 0), O(6, 1),
                           O(7, 0), O(7, 1), O(8, 0), O(8, 1),
                           O(9, 0), O(9, 1), O(10, 0), O(10, 1),
                           O(11, 0), O(11, 1)],
        }
        for t in range(NQT):
            attention(t, FILL[t])
        # last tile's out-projection: halves rotate through the freed score
        # slots' bank regions so nothing serializes on the copies
        potf = None
        for i, (qt, ncol) in enumerate(((12, 0), (13, 0), (12, 1), (14, 0),
                                        (13, 1), (15, 0), (14, 1), (15, 1))):
            if i % 2 == 0:
                potf = psp.tile([128, 1024], F32, name="ps", tag="ps", bufs=2)
            out_proj_half(qt, ncol, pot=potf, reg=i % 2)

    nc.finalize()
    return nc


_NC_CACHE = {}


def _get_nc():
    if "nc" not in _NC_CACHE:
        _NC_CACHE["nc"] = build_nc()
    return _NC_CACHE["nc"]


def kernel(x, Wqkv, bqkv, Wout, bout):
    x = np.asarray(x, dtype=np.float32)
    Wqkv = np.asarray(Wqkv, dtype=np.float32)
    bqkv = np.asarray(bqkv, dtype=np.float32)
    Wout = np.asarray(Wout, dtype=np.float32)
    bout = np.asarray(bout, dtype=np.float32)
    B, S_, D_ = x.shape
    assert (B, S_, D_) == (1, S, D)
    nc = _get_nc()

    xT_np = np.ascontiguousarray(x[0].T).astype(NPBF16)          # [768, 4096]
    in_maps = []
    for c in range(8):
        g, p = c // 2, c % 2
        csl = slice(DL * g, DL * (g + 1))
        rr = np.arange(128, dtype=np.int64)[:, None]
        cc = np.arange(64, dtype=np.int64)[None, :]
        mask = (rr <= 2 * cc + p).astype(NPBF16)
        bk_h = bqkv[D + DL * g:D + DL * (g + 1)].astype(np.float32)
        bq_h = bqkv[csl].astype(np.float32)
        in_maps.append({
            "xT": xT_np,
            "xqT": np.ascontiguousarray(xT_np[:, p::2]),
            "wk": np.ascontiguousarray(Wqkv[:, D + DL * g:D + DL * (g + 1)]).astype(NPBF16),
            "wq": np.ascontiguousarray(Wqkv[:, csl]).astype(NPBF16),
            "wv": np.ascontiguousarray(Wqkv[:, 2 * D + DL * g:2 * D + DL * (g + 1)]).astype(NPBF16),
            "bkq": np.ascontiguousarray(np.stack([bk_h, bq_h], axis=1)),
            "bv": np.ascontiguousarray(bqkv[2 * D + DL * g:2 * D + DL * (g + 1)]).astype(np.float32),
            "wout": np.ascontiguousarray(Wout[csl, :]).astype(NPBF16),
            "mask64": mask,
        })

    trace = bool(int(os.environ.get("ATTN_TRACE", "0")))
    tmpdir = os.environ.get("ATTN_TMPDIR") or None
    res = run_bass_kernel_spmd(nc, in_maps, core_ids=list(range(8)), trace=trace,
                               tmpdir=tmpdir)
    if trace:
        _NC_CACHE["last_result"] = res

    out_full = np.zeros((S, D), np.float32)
    for p in range(2):
        acc = np.zeros((SQ, D), np.float32)
        for g in range(4):
            acc += res.results[2 * g + p]["out"].astype(np.float32)
        out_full[p::2] = acc
    out_full += bout.astype(np.float32)[None, :]
    return out_full[None].astype(np.float32)


# revision 34
# speedup vs baseline: 1.1894x; 1.0353x over previous
"""Causal self-attention (B=1, S=4096, D=768, H=12) on 8 TRN2 NeuronCores.

Sharding: 4 head-groups (3 heads each) x 2 query-parity halves; no
collectives. Core c = 2*g + p handles heads [3g, 3g+3) and query rows
{r : r % 2 == p} (strided assignment balances causal work perfectly).

V3 highlights:
  - Score matmuls (contract dim 64 = half the PE) alternate the two
    64-row PE tiles: heads 0/1 live in opposite SBUF partition halves
    and head 2's K^T/Q^T are duplicated into both halves. Alternating
    row tiles dual-streams the PE (~108 ns per 512-col matmul vs 216,
    and same-tile back-to-back runs measure 427).
  - One EXP activation covers 2048 score cols (4 PSUM banks) - the
    ACT engine has ~300ns fixed cost per call.
  - Causal band truncation: diagonal blocks only compute their valid
    query range; a single [128,64] triangle mask is applied in-place.
  - Minimal head phase; K(2..7)/V(8..31)/Q(1..3)/out-proj all run as
    fillers inside the EXP-paced attention stream so the PE never
    idles (keeps the HAM clock at 2.4 GHz).

All matmuls run in bf16 (f32 PSUM accumulation); softmax exp in f32.
"""
import os

import numpy as np
import ml_dtypes

import concourse.bass as bass
import concourse.mybir as mybir
import concourse.tile as tile
from concourse import bacc
from concourse.bass_utils import run_bass_kernel_spmd

BF16 = mybir.dt.bfloat16
F32 = mybir.dt.float32
NPBF16 = ml_dtypes.bfloat16

S = 4096          # sequence length
D = 768           # model dim
HD = 64           # head dim
HL = 3            # heads per core
DL = HL * HD      # 192 local qkv cols per core
SQ = S // 2       # 2048 local queries per core
NQT = 4           # q-tiles per core
QTW = 512         # q-tile width (local queries)
NKB = S // 128    # 32 key blocks of 128
NDC = D // 128    # 6 contraction chunks of 128 over D
VW = HD + 1       # V' column stride per head (64 V cols + ones col)
SCALE = HD ** -0.5

# band packs: diagonal blocks b paired so each pack's widths sum to <=512
BAND_PACKS = ((0,), (1, 7), (2, 6), (3, 5), (4,))


def build_nc():
    nc = bacc.Bacc(None, target_bir_lowering=False)
    xT = nc.declare_dram_parameter("xT", [D, S], BF16, isOutput=False)
    xqT = nc.declare_dram_parameter("xqT", [D, SQ], BF16, isOutput=False)
    wk = nc.declare_dram_parameter("wk", [D, DL], BF16, isOutput=False)
    wq = nc.declare_dram_parameter("wq", [D, DL], BF16, isOutput=False)
    wv = nc.declare_dram_parameter("wv", [D, DL], BF16, isOutput=False)
    bkq = nc.declare_dram_parameter("bkq", [DL, 2], F32, isOutput=False)
    bv = nc.declare_dram_parameter("bv", [DL], F32, isOutput=False)
    wout = nc.declare_dram_parameter("wout", [DL, D], BF16, isOutput=False)
    mask64 = nc.declare_dram_parameter("mask64", [128, 64], BF16, isOutput=False)
    out = nc.declare_dram_parameter("out", [SQ, D], BF16, isOutput=True)

    from contextlib import ExitStack

    with tile.TileContext(nc) as tc, ExitStack() as ctx:
        persist = ctx.enter_context(tc.tile_pool(name="persist", bufs=1))
        xtp = ctx.enter_context(tc.tile_pool(name="xtp", bufs=1))
        wp = ctx.enter_context(tc.tile_pool(name="wp", bufs=1))
        pjp = ctx.enter_context(tc.tile_pool(name="pjp", bufs=1, space="PSUM"))
        psp = ctx.enter_context(tc.tile_pool(name="psp", bufs=1, space="PSUM"))
        pop = ctx.enter_context(tc.tile_pool(name="pop", bufs=2, space="PSUM"))
        ep = ctx.enter_context(tc.tile_pool(name="ep", bufs=2))
        rp = ctx.enter_context(tc.tile_pool(name="rp", bufs=2))
        osb = ctx.enter_context(tc.tile_pool(name="osb", bufs=3))

        kT01 = persist.tile([128, S], BF16)         # K^T heads 0,1
        kT2 = persist.tile([128, S], BF16)          # K^T head 2 (both halves)
        qT01 = persist.tile([128, SQ], BF16)        # Q^T heads 0,1
        qT2 = persist.tile([128, SQ], BF16)         # Q^T head 2 (both halves)
        aT01 = persist.tile([128, SQ], BF16)        # attn^T heads 0,1
        aT2 = persist.tile([64, SQ], BF16)
        vbig = persist.tile([128, NKB * HL * VW], BF16)  # V' blocks [k,195]
        bvb = persist.tile([128, DL], F32)          # bv broadcast over rows
        msk = persist.tile([128, 64], BF16)         # causal triangle r<=2c+p
        ones1 = persist.tile([1, 64], BF16)
        bkq0 = persist.tile([128, 2], F32)
        bkq1 = persist.tile([128, 2], F32)          # [0:64] and [64:128] same
        wo0 = persist.tile([128, D], BF16)
        wo1 = persist.tile([64, D], BF16)

        nc.gpsimd.memset(vbig, 1.0)
        nc.gpsimd.memset(ones1, 1.0)

        # x^T / xq^T land as 512-column slices holding all 6 contraction
        # chunks: tile cols = kc*512 + j. Weights land as [128, 6*DL].
        xt = [xtp.tile([128, NDC * 512], BF16, name=f"xt{n}") for n in range(8)]
        xq = [xtp.tile([128, NDC * 512], BF16, name=f"xq{t}") for t in range(NQT)]
        wk_t = wp.tile([128, NDC * DL], BF16, name="wk")
        wq_t = wp.tile([128, NDC * DL], BF16, name="wq")
        wv_t = wp.tile([128, NDC * DL], BF16, name="wv")

        xT_r = xT.rearrange("(c p) n -> p c n", p=128)      # [128, 6, 4096]
        xq_r = xqT.rearrange("(c p) n -> p c n", p=128)     # [128, 6, 2048]

        def dma_x(dst, src_r, j0):
            nc.sync.dma_start(
                out=dst.rearrange("p (c n) -> p c n", n=512),
                in_=src_r[:, :, j0:j0 + 512])

        # need-ordered input DMAs (sync queue ~0.7us issue each):
        nc.sync.dma_start(out=wk_t.rearrange("p (c m) -> p c m", m=DL),
                          in_=wk.rearrange("(c p) m -> p c m", p=128))
        dma_x(xt[0], xT_r, 0)
        dma_x(xt[1], xT_r, 512)
        nc.sync.dma_start(out=wv_t.rearrange("p (c m) -> p c m", m=DL),
                          in_=wv.rearrange("(c p) m -> p c m", p=128))
        nc.sync.dma_start(out=wq_t.rearrange("p (c m) -> p c m", m=DL),
                          in_=wq.rearrange("(c p) m -> p c m", p=128))
        dma_x(xq[0], xq_r, 0)
        dma_x(xt[2], xT_r, 1024)
        dma_x(xt[3], xT_r, 1536)
        dma_x(xq[1], xq_r, 512)
        dma_x(xt[4], xT_r, 2048)
        dma_x(xt[5], xT_r, 2560)
        dma_x(xq[2], xq_r, 1024)
        dma_x(xt[6], xT_r, 3072)
        dma_x(xt[7], xT_r, 3584)
        dma_x(xq[3], xq_r, 1536)
        # small tensors on the gpsimd queue (parallel issue path)
        nc.gpsimd.dma_start(out=bkq0, in_=bkq[0:128, :])
        nc.gpsimd.dma_start(out=bkq1[0:64, :], in_=bkq[128:DL, :])
        nc.gpsimd.dma_start(out=bkq1[64:128, :], in_=bkq[128:DL, :])
        nc.gpsimd.dma_start(out=bvb, in_=bv[:].partition_broadcast(128))
        nc.gpsimd.dma_start(out=msk, in_=mask64[:, :])
        nc.gpsimd.dma_start(out=wo0, in_=wout[0:128, :])
        nc.gpsimd.dma_start(out=wo1, in_=wout[128:DL, :])

        def kq_proj(dst01, dst2, w_t, rhs, bc, n, m, head=False):
            # dst[m-rows, cols n*512..] = W^T x^T + b  for one m-pass.
            # head=True: rotate through the (idle) score PSUM slots and
            # evacuate on the (idle) scalar engine - no single-slot WAR
            # stall, no DVE backlog.
            nsl = slice(n * 512, (n + 1) * 512)
            mw = 128 if m == 0 else 64
            msl = slice(0, 128) if m == 0 else slice(128, DL)
            if head:
                ps = psp.tile([128, 1024], F32, name="ps", tag="ps",
                              bufs=2)[:, 0:512]
            else:
                ps = pjp.tile([128, 512], F32, name="pj", tag="pj")
            for kc in range(NDC):
                nc.tensor.matmul(
                    ps[:mw, :],
                    lhsT=w_t[:, kc * DL:(kc + 1) * DL][:, msl],
                    rhs=rhs[:, kc * 512:(kc + 1) * 512],
                    start=(kc == 0), stop=(kc == NDC - 1),
                )
            ident = mybir.ActivationFunctionType.Identity

            def evac(dst, src, bias):
                if head:
                    nc.scalar.activation(out=dst, in_=src, func=ident,
                                         bias=bias)
                else:
                    nc.vector.tensor_scalar_add(out=dst, in0=src,
                                                scalar1=bias)

            if m == 0:
                evac(dst01[:, nsl], ps, bkq0[:, bc:bc + 1])
            else:  # head 2: write both partition halves (dual-tile scores)
                evac(dst2[0:64, nsl], ps[:64, :], bkq1[0:64, bc:bc + 1])
                evac(dst2[64:128, nsl], ps[:64, :], bkq1[64:128, bc:bc + 1])

        def v_proj(kb, head=False):
            if head:
                pv = psp.tile([128, 1024], F32, name="ps", tag="ps",
                              bufs=2)[:, 0:512]
            else:
                pv = pjp.tile([128, 512], F32, name="pj", tag="pj")
            n, j = kb // 4, (kb % 4) * 128
            for kc in range(NDC):
                nc.tensor.matmul(
                    pv[:, :DL], lhsT=xt[n][:, kc * 512 + j:kc * 512 + j + 128],
                    rhs=wv_t[:, kc * DL:(kc + 1) * DL],
                    start=(kc == 0), stop=(kc == NDC - 1),
                )
            # one strided add writes all 3 heads' V cols (ones col skipped)
            voff = kb * HL * VW
            dstv = vbig[:, voff:voff + HL * VW]
            dstv = dstv.rearrange("p (h vw) -> p h vw", vw=VW)[:, :, 0:HD]
            nc.vector.tensor_add(
                out=dstv,
                in0=pv[:, :DL].rearrange("p (h d) -> p h d", d=HD),
                in1=bvb.rearrange("p (h d) -> p h d", d=HD),
            )

        # per-head score/attn tiles: (lhsT source, rhs source, aT dest)
        kq_src = (
            (kT01, qT01, (0, 64)),     # head 0: always low half
            (kT01, qT01, (64, 128)),   # head 1: always high half
            (kT2, qT2, None),          # head 2: half chosen per matmul
        )
        aT_of = (aT01[0:64], aT01[64:128], aT2)

        ot_tiles = {}

        def out_proj_half(qt, ncol, pot=None, reg=0, act_copy=False):
            # split at the PSUM-slot reuse boundary so the WAR wait on the
            # previous half's copy never stalls the PE mid-filler
            osl = slice(qt * 128, (qt + 1) * 128)
            if ncol == 0:
                ot_tiles[qt] = osb.tile([128, D], BF16, name="ot", tag="ot")
            ot = ot_tiles[qt]
            cw = 512 if ncol == 0 else 256
            csl = slice(ncol * 512, ncol * 512 + cw)
            final = pot is not None
            if not final:
                pot = pjp.tile([128, 512], F32, name="pj", tag="pj")
            psl = slice(reg * 512, reg * 512 + cw)
            nc.tensor.matmul(
                pot[:, psl], lhsT=aT01[:, osl], rhs=wo0[:, csl],
                start=True, stop=False, skip_group_check=True)
            nc.tensor.matmul(
                pot[:, psl], lhsT=aT2[:, osl], rhs=wo1[:, csl],
                start=False, stop=True, skip_group_check=True)
            if act_copy:  # tail: split evacuations across DVE and ACT
                nc.scalar.activation(out=ot[:, csl], in_=pot[:, psl],
                                     func=mybir.ActivationFunctionType.Copy)
            else:
                nc.vector.tensor_copy(out=ot[:, csl], in_=pot[:, psl])
            if final:
                nc.gpsimd.dma_start(out=out[osl, csl], in_=ot[:, csl])
            elif ncol == 1:
                nc.gpsimd.dma_start(out=out[osl, :], in_=ot)

        def attention(t, fillers):
            def pump(k=1):
                for _ in range(k):
                    if fillers:
                        fillers.pop(0)()

            qoff = t * QTW
            last_kb = 8 * t + BAND_PACKS[-1][-1]

            # entry = (half, head, kb, psum_off, width, q_start, band)
            # pack = (entries, exp_ranges, heads_finishing)
            packs = []
            # phase A: heads 0+1 paired on alternating PE row tiles
            for kb in range(0, 8 * t):
                packs.append((
                    [(0, 0, kb, 0, 512, 0, False),
                     (64, 1, kb, 512, 512, 0, False)],
                    ((0, 1024),), ()))
            for pr in BAND_PACKS[:-1]:
                ent = []
                offs = [0, 512]
                for b in pr:
                    w = 512 - 64 * b
                    for h in (0, 1):
                        ent.append((64 * h, h, 8 * t + b, offs[h], w,
                                    64 * b, True))
                        offs[h] += w
                packs.append((ent, ((0, 1024),), ()))
            packs.append((
                [(0, 0, 8 * t + 4, 0, 256, 256, True),
                 (64, 1, 8 * t + 4, 512, 256, 256, True)],
                ((0, 256), (512, 768)), (0, 1)))
            # phase B: head 2 alternating its two duplicated halves
            for kb in range(0, 8 * t, 2):
                packs.append((
                    [(0, 2, kb, 0, 512, 0, False),
                     (64, 2, kb + 1, 512, 512, 0, False)],
                    ((0, 1024),), ()))
            for i, prs in enumerate((((0,), (1, 7)), ((2, 6), (3, 5)))):
                ent = []
                for j, pr in enumerate(prs):
                    off = 512 * j
                    for b in pr:
                        w = 512 - 64 * b
                        ent.append((64 * ((2 * i + j) % 2), 2, 8 * t + b,
                                    off, w, 64 * b, True))
                        off += w
                packs.append((ent, ((0, 1024),), ()))
            packs.append((
                [(0, 2, 8 * t + 4, 0, 256, 256, True)],
                ((0, 256),), (2,)))

            po_of = {}

            def emit_pv(pack, eT):
                entries, _, fin = pack
                for (_half, h, kb, off, w, qs, _band) in entries:
                    if h not in po_of:
                        # lazy: the slot's previous reader (divide of the
                        # evicted head) must already be emitted for the WAR
                        po_of[h] = pop.tile([VW, 512], F32, name="po",
                                            tag="po")
                    voff = kb * HL * VW + h * VW
                    nc.tensor.matmul(
                        po_of[h][0:VW, qs:qs + w],
                        lhsT=vbig[:, voff:voff + VW],
                        rhs=eT[:, off:off + w],
                        start=(kb == 0), stop=(kb == last_kb),
                        skip_group_check=True,
                    )
                for h in fin:
                    divide(h)

            def divide(h):
                # divide by the softmax sum (row HD of po)
                po = po_of[h]
                sums = rp.tile([1, 512], BF16, name="sums", tag="sums")
                nc.vector.tensor_copy(out=sums, in_=po[HD:VW, :])
                pb = pjp.tile([128, 512], F32, name="pj", tag="pj")
                nc.tensor.matmul(pb[0:64, :], lhsT=ones1, rhs=sums,
                                 start=True, stop=True)
                recb = rp.tile([64, 512], F32, name="recb", tag="recb")
                nc.vector.reciprocal_approx_fast(out=recb, in_=pb[0:64, :])
                nc.vector.tensor_mul(
                    out=aT_of[h][:, qoff:qoff + QTW], in0=po[0:HD, :],
                    in1=recb)

            pend = None  # (pack, eT) whose PV is not yet emitted
            for pack in packs:
                entries, exp_ranges, _fin = pack
                ps = psp.tile([128, 1024], F32, name="ps", tag="ps", bufs=2)
                for (half, h, kb, off, w, qs, _band) in entries:
                    kT_h, qT_h, fixed = kq_src[h]
                    if fixed is not None:
                        hsl = slice(fixed[0], fixed[1])
                    else:
                        hsl = slice(half, half + 64)
                    nc.tensor.matmul(
                        ps[:, off:off + w],
                        lhsT=kT_h[hsl, kb * 128:(kb + 1) * 128],
                        rhs=qT_h[hsl, qoff + qs:qoff + QTW],
                        start=True, stop=True,
                    )
                eT = ep.tile([128, 1024], BF16, name="eT", tag="eT", bufs=3)
                for (r0, r1) in exp_ranges:
                    nc.scalar.activation(
                        out=eT[:, r0:r1], in_=ps[:, r0:r1],
                        func=mybir.ActivationFunctionType.Exp, scale=SCALE)
                for (_half, _h, kb, off, w, qs, band) in entries:
                    if band:  # zero the 64 partial cols of the triangle
                        nc.vector.tensor_mul(
                            out=eT[:, off:off + 64],
                            in0=eT[:, off:off + 64], in1=msk)
                pump(1)
                if pend is not None:
                    emit_pv(*pend)
                pend = (pack, eT)
            emit_pv(*pend)
            pump(len(fillers))

        # ---- schedule: minimal head, then q-tiles t=0..3 with fillers ----
        def K(n, m):
            return lambda: kq_proj(kT01, kT2, wk_t, xt[n], 0, n, m)

        def Q(t, m):
            return lambda: kq_proj(qT01, qT2, wq_t, xq[t], 1, t, m)

        def V(kb):
            return lambda: v_proj(kb)

        def O(qt, ncol):
            return lambda: out_proj_half(qt, ncol)

        # head: only what attention(0)'s first packs need, DMA-ordered
        kq_proj(kT01, kT2, wk_t, xt[0], 0, 0, 0, head=True)
        kq_proj(kT01, kT2, wk_t, xt[0], 0, 0, 1, head=True)
        v_proj(0, head=True)
        v_proj(1, head=True)
        kq_proj(kT01, kT2, wk_t, xt[1], 0, 1, 0, head=True)
        kq_proj(kT01, kT2, wk_t, xt[1], 0, 1, 1, head=True)
        v_proj(7, head=True)
        kq_proj(qT01, qT2, wq_t, xq[0], 1, 0, 0, head=True)
        kq_proj(qT01, qT2, wq_t, xq[0], 1, 0, 1, head=True)

        # V fillers ordered by the band-block usage order of the next tile
        def Vband(t):
            return [V(8 * t + b) for b in (0, 1, 7, 2, 6, 3, 5, 4)]

        FILL = {
            0: [V(2), V(6), V(3), V(5), V(4), K(2, 0), K(2, 1),
                K(3, 0), K(3, 1), Q(1, 0), Q(1, 1)],
            1: Vband(1) + [K(4, 0), K(4, 1), K(5, 0), K(5, 1),
                           Q(2, 0), Q(2, 1)],
            2: Vband(2) + [K(6, 0), K(6, 1), K(7, 0), K(7, 1),
                           Q(3, 0), Q(3, 1), O(0, 0), O(0, 1)],
            3: Vband(3) + [O(1, 0), O(1, 1), O(2, 0), O(2, 1),
                           O(3, 0), O(3, 1), O(4, 0), O(4, 1),
                           O(5, 0), O(5, 1), O(6, 0), O(6, 1),
                           O(7, 0), O(7, 1), O(8, 0), O(8, 1),
                           O(9, 0), O(9, 1), O(10, 0), O(10, 1),
                           O(11, 0), O(11, 1)],
        }
        for t in range(NQT):
            attention(t, FILL[t])
        # last tile's out-projection: halves rotate through the freed score
        # slots' bank regions so nothing serializes on the copies
        potf = None
        for i, (qt, ncol) in enumerate(((12, 0), (13, 0), (12, 1), (14, 0),
                                        (13, 1), (15, 0), (14, 1), (15, 1))):
            if i % 2 == 0:
                potf = psp.tile([128, 1024], F32, name="ps", tag="ps", bufs=2)
            out_proj_half(qt, ncol, pot=potf, reg=i % 2, act_copy=bool(i % 2))

    nc.finalize()
    return nc


_NC_CACHE = {}


def _get_nc():
    if "nc" not in _NC_CACHE:
        _NC_CACHE["nc"] = build_nc()
    return _NC_CACHE["nc"]


def kernel(x, Wqkv, bqkv, Wout, bout):
    x = np.asarray(x, dtype=np.float32)
    Wqkv = np.asarray(Wqkv, dtype=np.float32)
    bqkv = np.asarray(bqkv, dtype=np.float32)
    Wout = np.asarray(Wout, dtype=np.float32)
    bout = np.asarray(bout, dtype=np.float32)
    B, S_, D_ = x.shape
    assert (B, S_, D_) == (1, S, D)
    nc = _get_nc()

    xT_np = np.ascontiguousarray(x[0].T).astype(NPBF16)          # [768, 4096]
    in_maps = []
    for c in range(8):
        g, p = c // 2, c % 2
        csl = slice(DL * g, DL * (g + 1))
        rr = np.arange(128, dtype=np.int64)[:, None]
        cc = np.arange(64, dtype=np.int64)[None, :]
        mask = (rr <= 2 * cc + p).astype(NPBF16)
        bk_h = bqkv[D + DL * g:D + DL * (g + 1)].astype(np.float32)
        bq_h = bqkv[csl].astype(np.float32)
        in_maps.append({
            "xT": xT_np,
            "xqT": np.ascontiguousarray(xT_np[:, p::2]),
            "wk": np.ascontiguousarray(Wqkv[:, D + DL * g:D + DL * (g + 1)]).astype(NPBF16),
            "wq": np.ascontiguousarray(Wqkv[:, csl]).astype(NPBF16),
            "wv": np.ascontiguousarray(Wqkv[:, 2 * D + DL * g:2 * D + DL * (g + 1)]).astype(NPBF16),
            "bkq": np.ascontiguousarray(np.stack([bk_h, bq_h], axis=1)),
            "bv": np.ascontiguousarray(bqkv[2 * D + DL * g:2 * D + DL * (g + 1)]).astype(np.float32),
            "wout": np.ascontiguousarray(Wout[csl, :]).astype(NPBF16),
            "mask64": mask,
        })

    trace = bool(int(os.environ.get("ATTN_TRACE", "0")))
    tmpdir = os.environ.get("ATTN_TMPDIR") or None
    res = run_bass_kernel_spmd(nc, in_maps, core_ids=list(range(8)), trace=trace,
                               tmpdir=tmpdir)
    if trace:
        _NC_CACHE["last_result"] = res

    out_full = np.zeros((S, D), np.float32)
    for p in range(2):
        acc = np.zeros((SQ, D), np.float32)
        for g in range(4):
            acc += res.results[2 * g + p]["out"].astype(np.float32)
        out_full[p::2] = acc
    out_full += bout.astype(np.float32)[None, :]
    return out_full[None].astype(np.float32)
